# revision 14
# baseline (speedup 1.0000x reference)
"""Causal self-attention (B=2, S=2048, D=2048, H=16) on 8 TRN2 NeuronCores.

Sharding: 2 batches x 4 head-groups.  Core c handles batch c//4 and heads
[4*(c%4) .. 4*(c%4)+3]; each core produces output columns [512*(c%4) ...].

Schedule (per core), tuned from the 447us-baseline trace:
  - DMA issue order interleaves (wv[dt], xT[dt, tb0]) so the first v-proj
    accumulation starts ~2us in (old coarse wait cost ~28us of prologue).
  - q-blocks ascend (0..3); AllGather per (head, token-half) fires after
    qb1 and qb3, so AGs spread evenly over the run (16 tiny per-qb AGs
    congested the serial CC stream at the tail; 8 is the sweet spot).
  - out-projection pass lags ONE head (not two): attn(h) -> qk(h+1) ->
    pass(h); the tail after attn(3) is just pass(3) (~17us) with its AGs
    already landed or landing during the pass.
  - softmax-denominator accumulation (esum) stays on DVE: the Pool
    engine's generic tensor_tensor is ~4x slower and its config switches
    stall partition_broadcast on the ynorm critical path.

Softmax uses exp without max subtraction (logits are O(8) here); denominators
are accumulated on DVE over k-tile pairs then reduced across partitions with
a single ones-matmul, inverted with reciprocal_approx_fast.

Compute is bf16 with fp32 PSUM accumulation; measured l2 rel err vs the fp32
reference is ~5.9e-3.
"""

import numpy as np
import ml_dtypes

B, S, D = 2, 2048, 2048
H, HD = 16, 128
HLOC = 4           # heads per core
CW = HLOC * HD     # 512: per-core v width and out-column width
QB = 4             # q blocks of 512
DT = 16            # d tiles of 128
TB = 4             # token blocks of 512
SCALE = 1.0 / float(np.sqrt(HD))
GROUPS = [[0, 1, 2, 3], [4, 5, 6, 7]]

_cache = {}


def _build():
    import concourse.tile as tile
    import concourse.mybir as mybir
    from concourse import bacc

    BF = mybir.dt.bfloat16
    F32 = mybir.dt.float32

    nc = bacc.Bacc("TRN2", target_bir_lowering=False, debug=False, num_devices=8)

    # Inputs (per-core shards, host-prepared)
    xT = nc.dram_tensor("xT", [D, S], BF, kind="ExternalInput")          # x[batch].T
    wqk = nc.dram_tensor("wqk", [HLOC, 2, DT, 128, 128], BF, kind="ExternalInput")
    wv = nc.dram_tensor("wv", [DT, 128, CW], BF, kind="ExternalInput")
    bqk = nc.dram_tensor("bqk", [HLOC, 2, 128, 1], F32, kind="ExternalInput")
    bv = nc.dram_tensor("bv", [1, CW], F32, kind="ExternalInput")
    # w_out rows permuted: wout[h][i] = w_out[512*i + 128*h : +128, cols]
    wout = nc.dram_tensor("wout", [HLOC, 4, 128, CW], BF, kind="ExternalInput")
    bout = nc.dram_tensor("bout", [1, CW], F32, kind="ExternalInput")
    out = nc.dram_tensor("out", [S, CW], F32, kind="ExternalOutput")

    # per (head, token-half) AG buffers; half hf covers tokens
    # [1024*hf, 1024*(hf+1)) = q-blocks {2hf, 2hf+1}
    ag_in = {(h, hf): nc.dram_tensor(f"ag_in{h}_{hf}", [128, 1024], BF,
                                     kind="Internal")
             for h in range(HLOC - 1) for hf in range(2)}
    ag_out = {(h, hf): nc.dram_tensor(f"ag_out{h}_{hf}", [512, 1024], BF,
                                      kind="Internal")
              for h in range(HLOC - 1) for hf in range(2)}
    # head 3 gathers per q-block so the critical last AG is small and early
    ag_in3 = {qb: nc.dram_tensor(f"ag_in3_{qb}", [128, 512], BF,
                                 kind="Internal") for qb in range(QB)}
    ag_out3 = {qb: nc.dram_tensor(f"ag_out3_{qb}", [512, 512], BF,
                                  kind="Internal") for qb in range(QB)}

    with tile.TileContext(nc) as tc:
        with (
            tc.tile_pool(name="const", bufs=1) as constp,
            tc.tile_pool(name="pers", bufs=1) as pers,
            tc.tile_pool(name="work", bufs=2) as work,
            tc.tile_pool(name="psum", bufs=2, space="PSUM") as psum,
        ):
            # ---- constants ----
            ones = constp.tile([128, 1], BF, name="ones")
            nc.gpsimd.memset(ones[:], 1.0)

            # Pair masks for the 4 diagonal k-subtiles, packed two subtiles
            # wide: pairmask[m][:, 512*sub + qq] keeps where
            # qq >= kk + 128*(2m+sub).
            pairmasks = []
            for pm in range(2):
                m = constp.tile([128, 1024], BF, name=f"pmask{pm}",
                                tag=f"pmask{pm}")
                nc.gpsimd.memset(m[:], 1.0)
                for sub in range(2):
                    nc.gpsimd.affine_select(
                        out=m[:, sub * 512:(sub + 1) * 512],
                        in_=m[:, sub * 512:(sub + 1) * 512],
                        compare_op=mybir.AluOpType.is_ge, fill=0.0,
                        base=-128 * (2 * pm + sub), channel_multiplier=-1,
                        pattern=[[1, 512]],
                    )
                pairmasks.append(m)

            bout_sb = constp.tile([1, CW], F32, name="bout_sb")
            nc.sync.dma_start(bout_sb[:], bout[:])
            bias_bc = constp.tile([128, CW], F32, name="bias_bc")
            nc.gpsimd.partition_broadcast(bias_bc[:], bout_sb[:], channels=128)

            bv_sb = constp.tile([1, CW], F32, name="bv_sb")
            nc.sync.dma_start(bv_sb[:], bv[:])
            vbias_bc = constp.tile([128, CW], F32, name="vbias_bc")
            nc.gpsimd.partition_broadcast(vbias_bc[:], bv_sb[:], channels=128)

            bqk_sb = {}
            for h in range(HLOC):
                for qk in range(2):
                    t = constp.tile([128, 1], F32, name=f"bqk{h}{qk}",
                                    tag=f"bqk{h}{qk}")
                    nc.sync.dma_start(t[:], bqk[h, qk])
                    bqk_sb[(h, qk)] = t

            # ---- persistent v tiles ----
            vt = [pers.tile([128, CW], BF, name=f"v{t}", tag=f"v{t}")
                  for t in range(16)]

            # ---- loads: (wv[dt], xT[dt,tb0]) interleaved so the first
            # v-proj accumulation chain is gated only on its own tiles,
            # then the remaining token blocks ----
            wv_sb = []
            xt_tiles = {}

            def load_xt(dt, tb):
                t = work.tile([128, 512], BF, name=f"xt_{dt}_{tb}",
                              tag="xT", bufs=68)
                nc.sync.dma_start(
                    t[:], xT[dt * 128:(dt + 1) * 128, tb * 512:(tb + 1) * 512])
                xt_tiles[(dt, tb)] = t

            for dt in range(DT):
                wvp = work.tile([128, CW], BF, name=f"wvp{dt}", tag="p512",
                                bufs=17)
                nc.sync.dma_start(wvp[:], wv[dt])
                wv_sb.append(wvp)
                load_xt(dt, 0)
            for tb in range(1, TB):
                for dt in range(DT):
                    load_xt(dt, tb)

            # ---- v projection: v[t] = x @ wv  ([tok, vcol], xT stationary) ----
            for t in range(16):
                tb, j = t // 4, t % 4
                acc = psum.tile([128, CW], F32, name="acc_v", tag="acc", bufs=2)
                for dt in range(DT):
                    nc.tensor.matmul(
                        acc[:],
                        xt_tiles[(dt, tb)][:, j * 128:(j + 1) * 128],
                        wv_sb[dt][:],
                        start=(dt == 0), stop=(dt == DT - 1),
                    )
                nc.vector.tensor_tensor(vt[t][:], acc[:], vbias_bc[:],
                                        mybir.AluOpType.add)

            # ---- per-head q/k projection ([col, tok] transposed) ----
            def qk_proj(h):
                dests = {}
                for qk in range(2):
                    wts = []
                    for dt in range(DT):
                        wt = work.tile([128, 128], BF, name=f"w_{h}_{qk}_{dt}",
                                       tag="w", bufs=24)
                        nc.sync.dma_start(wt[:], wqk[h, qk, dt])
                        wts.append(wt)
                    dest = work.tile([128, S], BF, name=f"qkT_{h}_{qk}",
                                     tag="qkT", bufs=4)
                    for tb in range(TB):
                        acc = psum.tile([128, 512], F32, name="acc_qk",
                                        tag="acc", bufs=2)
                        for dt in range(DT):
                            nc.tensor.matmul(
                                acc[:], wts[dt][:], xt_tiles[(dt, tb)][:],
                                start=(dt == 0), stop=(dt == DT - 1),
                            )
                        nc.scalar.activation(
                            dest[:, tb * 512:(tb + 1) * 512], acc[:],
                            mybir.ActivationFunctionType.Identity,
                            bias=bqk_sb[(h, qk)][:], scale=1.0,
                        )
                    dests[qk] = dest
                return dests[0], dests[1]

            # ---- attention for one head (q-blocks ASCENDING) + per-qb AG ----
            def attention_head(h, qTh, kTh):
                for qb in range(QB):
                    nk = 4 * qb + 4
                    npair = nk // 2
                    order = list(range(npair))
                    y_ps = psum.tile([128, 512], F32, name="y_ps", tag="y")
                    esum = work.tile([128, 1024], BF, name="esum", tag="esum",
                                     bufs=2)
                    prev = None
                    nflush = [0]

                    def flush(prev_pair):
                        e, pr = prev_pair
                        first = nflush[0] == 0
                        last = nflush[0] == npair - 1
                        trimmed = pr == npair - 1
                        for s_ in range(2):
                            kt = 2 * pr + s_
                            off = 128 * (2 + s_) if trimmed else 0
                            nc.tensor.matmul(
                                y_ps[:, off:512],
                                vt[kt][:, h * 128:(h + 1) * 128],
                                e[:, s_ * 512 + off:(s_ + 1) * 512],
                                start=(first and s_ == 0),
                                stop=(last and s_ == 1),
                                skip_group_check=trimmed,
                            )
                        if first:
                            nc.vector.tensor_copy(esum[:], e[:])
                        elif trimmed:
                            for s_ in range(2):
                                off = 128 * (2 + s_)
                                sl = slice(s_ * 512 + off, (s_ + 1) * 512)
                                nc.vector.tensor_tensor(
                                    esum[:, sl], esum[:, sl], e[:, sl],
                                    mybir.AluOpType.add)
                        else:
                            nc.vector.tensor_tensor(esum[:], esum[:], e[:],
                                                    mybir.AluOpType.add)
                        nflush[0] += 1

                    for pr in order:
                        # the last pair of each q-block covers k-subtiles 2,3
                        # of the diagonal superblock: only q >= 128*ks is
                        # causally live, so compute the trapezoid only
                        trimmed = pr == npair - 1
                        sc = psum.tile([128, 1024], F32, name="sc", tag="s",
                                       bufs=2)
                        for s_ in range(2):
                            kt = 2 * pr + s_
                            off = 128 * (2 + s_) if trimmed else 0
                            nc.tensor.matmul(
                                sc[:, s_ * 512 + off:(s_ + 1) * 512],
                                kTh[:, kt * 128:(kt + 1) * 128],
                                qTh[:, qb * 512 + off:(qb + 1) * 512],
                                start=True, stop=True,
                            )
                        e = work.tile([128, 1024], BF, name="expT", tag="expT",
                                      bufs=4)
                        if trimmed:
                            for s_ in range(2):
                                off = 128 * (2 + s_)
                                sl = slice(s_ * 512 + off, (s_ + 1) * 512)
                                nc.scalar.activation(
                                    e[:, sl], sc[:, sl],
                                    mybir.ActivationFunctionType.Exp,
                                    scale=SCALE,
                                )
                                nc.vector.tensor_tensor(
                                    e[:, sl], e[:, sl], pairmasks[1][:, sl],
                                    mybir.AluOpType.mult)
                        else:
                            nc.scalar.activation(
                                e[:], sc[:], mybir.ActivationFunctionType.Exp,
                                scale=SCALE,
                            )
                            if pr == npair - 2:
                                nc.vector.tensor_tensor(e[:], e[:],
                                                        pairmasks[0][:],
                                                        mybir.AluOpType.mult)
                        if prev is not None:
                            flush(prev)
                        prev = (e, pr)
                    flush(prev)

                    esum_f = work.tile([128, 512], BF, name="esum_f",
                                       tag="esum_f", bufs=2)
                    nc.vector.tensor_tensor(esum_f[:], esum[:, 0:512],
                                            esum[:, 512:1024],
                                            mybir.AluOpType.add)
                    sum_ps = psum.tile([1, 512], F32, name="sum_ps", tag="y")
                    nc.tensor.matmul(sum_ps[:], ones[:], esum_f[:],
                                     start=True, stop=True)
                    recip = work.tile([1, 512], F32, name="recip", tag="recip",
                                      bufs=2)
                    nc.vector.reciprocal_approx_fast(recip[:], sum_ps[:])
                    rbc = work.tile([128, 512], F32, name="rbc", tag="rbc",
                                    bufs=2)
                    nc.gpsimd.partition_broadcast(rbc[:], recip[:], channels=128)
                    ynorm = work.tile([128, 512], BF, name="ynorm", tag="ynorm",
                                      bufs=3)
                    nc.vector.tensor_tensor(ynorm[:], y_ps[:], rbc[:],
                                            mybir.AluOpType.mult)
                    if h == HLOC - 1:
                        nc.sync.dma_start(ag_in3[qb][:], ynorm[:])
                        nc.gpsimd.collective_compute(
                            "AllGather", mybir.AluOpType.bypass,
                            replica_groups=GROUPS,
                            ins=[ag_in3[qb].ap()],
                            outs=[ag_out3[qb].ap()],
                        )
                    else:
                        hf, co = qb // 2, (qb % 2) * 512
                        nc.sync.dma_start(
                            ag_in[(h, hf)][:, co:co + 512], ynorm[:])
                        if qb in (1, 3):
                            nc.gpsimd.collective_compute(
                                "AllGather", mybir.AluOpType.bypass,
                                replica_groups=GROUPS,
                                ins=[ag_in[(h, hf)].ap()],
                                outs=[ag_out[(h, hf)].ap()],
                            )

            # ---- out-projection partial pass for head-chunk h ----
            wout_sb = {}

            def load_wout():
                for h in range(HLOC):
                    for i in range(4):
                        t = work.tile([128, CW], BF, name=f"wout{h}{i}",
                                      tag="p512", bufs=17)
                        nc.sync.dma_start(t[:], wout[h, i])
                        wout_sb[(h, i)] = t

            part = {}

            def outproj_pass(h):
                for tc_ in range(4):
                    hf, co = tc_ // 2, (tc_ % 2) * 512
                    ygt = []
                    for i in range(4):
                        t = work.tile([128, 512], BF, name=f"yg_{h}_{tc_}_{i}",
                                      tag="ygt", bufs=10)
                        if h == HLOC - 1:
                            src = ag_out3[tc_][i * 128:(i + 1) * 128, :]
                        else:
                            src = ag_out[(h, hf)][i * 128:(i + 1) * 128,
                                                  co:co + 512]
                        nc.sync.dma_start(t[:], src)
                        ygt.append(t)
                    for j in range(4):
                        t = tc_ * 4 + j
                        acc = psum.tile([128, CW], F32, name="acc_o",
                                        tag="acc", bufs=2)
                        for i in range(4):
                            nc.tensor.matmul(
                                acc[:],
                                ygt[i][:, j * 128:(j + 1) * 128],
                                wout_sb[(h, i)][:],
                                start=(i == 0), stop=(i == 3),
                            )
                        if h == 0:
                            p = work.tile([128, CW], BF, name=f"part{t}",
                                          tag=f"part{t}", bufs=1)
                            part[t] = p
                            nc.vector.tensor_tensor(p[:], acc[:], bias_bc[:],
                                                    mybir.AluOpType.add)
                        elif h < HLOC - 1:
                            nc.vector.tensor_tensor(part[t][:], part[t][:],
                                                    acc[:],
                                                    mybir.AluOpType.add)
                        else:
                            osb = work.tile([128, CW], F32, name="osb",
                                            tag="osb", bufs=3)
                            nc.vector.tensor_tensor(osb[:], part[t][:], acc[:],
                                                    mybir.AluOpType.add)
                            nc.sync.dma_start(
                                out[t * 128:(t + 1) * 128, :], osb[:])

            # ---- head pipeline, pass lags ONE head behind attention ----
            qk_tiles = qk_proj(0)
            load_wout()
            for h in range(HLOC):
                attention_head(h, *qk_tiles)
                qk_tiles = qk_proj(h + 1) if h + 1 < HLOC else None
                if h > 0:
                    outproj_pass(h - 1)
            outproj_pass(HLOC - 1)

    nc.compile()
    return nc


def _prep_inputs(x, w_qkv, b_qkv, w_out, b_out):
    """Host-side sharding/layout. Returns in_maps for the 8 cores."""
    bf16 = ml_dtypes.bfloat16
    x = np.asarray(x, dtype=np.float32)
    w_qkv = np.asarray(w_qkv, dtype=np.float32)
    b_qkv = np.asarray(b_qkv, dtype=np.float32)
    w_out = np.asarray(w_out, dtype=np.float32)
    b_out = np.asarray(b_out, dtype=np.float32)

    xT_b = [np.ascontiguousarray(x[b].T).astype(bf16) for b in range(B)]

    in_maps = []
    for c in range(8):
        b, g = c // 4, c % 4
        cols = slice(CW * g, CW * (g + 1))

        # wqk[h][0]=q, [1]=k tiles for global head 4g+h, [dt, 128, 128]
        wqk = np.empty((HLOC, 2, DT, 128, 128), np.float32)
        bqk = np.empty((HLOC, 2, 128, 1), np.float32)
        for h in range(HLOC):
            gh = 4 * g + h
            for qk in range(2):
                wcol = w_qkv[:, qk * D + 128 * gh: qk * D + 128 * (gh + 1)]
                wqk[h, qk] = wcol.reshape(DT, 128, 128)
                bqk[h, qk, :, 0] = b_qkv[qk * D + 128 * gh: qk * D + 128 * (gh + 1)]

        wv_ = w_qkv[:, 2 * D:3 * D][:, cols]
        bv_ = b_qkv[2 * D:3 * D][cols]

        # w_out rows permuted to the AG's rank-major order per head chunk
        wout_loc = w_out[:, cols]
        wout_t = np.empty((HLOC, 4, 128, CW), np.float32)
        for h in range(HLOC):
            for i in range(4):
                wout_t[h, i] = wout_loc[512 * i + 128 * h: 512 * i + 128 * (h + 1), :]

        in_maps.append({
            "xT": xT_b[b],
            "wqk": np.ascontiguousarray(wqk).astype(bf16),
            "wv": np.ascontiguousarray(wv_.reshape(DT, 128, CW)).astype(bf16),
            "bqk": np.ascontiguousarray(bqk),
            "bv": np.ascontiguousarray(bv_.reshape(1, CW)),
            "wout": np.ascontiguousarray(wout_t).astype(bf16),
            "bout": np.ascontiguousarray(b_out[cols].reshape(1, CW)),
        })
    return in_maps


def kernel(x, w_qkv, b_qkv, w_out, b_out, _trace=False, _trace_kwargs=None):
    from concourse.bass_utils import run_bass_kernel_spmd

    if "nc" not in _cache:
        _cache["nc"] = _build()
    nc = _cache["nc"]

    in_maps = _prep_inputs(x, w_qkv, b_qkv, w_out, b_out)
    res = run_bass_kernel_spmd(
        nc, in_maps, core_ids=list(range(8)),
        trace=_trace, **(_trace_kwargs or {}),
    )

    out = np.empty((B, S, D), dtype=np.float32)
    for c in range(8):
        b, g = c // 4, c % 4
        out[b][:, CW * g:CW * (g + 1)] = res.results[c]["out"]
    kernel.last_result = res
    return out


# revision 18
# speedup vs baseline: 1.0231x; 1.0231x over previous
"""Causal self-attention (B=2, S=2048, D=2048, H=16) on 8 TRN2 NeuronCores.

Sharding: 2 batches x 4 head-groups.  Core c handles batch c//4 and heads
[4*(c%4) .. 4*(c%4)+3]; each core produces output columns [512*(c%4) ...].

Schedule (per core), tuned from the 447us-baseline trace:
  - weights/activations are host-packed partition-major so each input loads
    with a handful of wide DMAs (each DMA_DIRECT2D costs ~0.6us of serial
    Sync-engine descgen; the old per-tile loads burned ~200us of Sync time
    and ~10us of prologue).
  - q-blocks ascend (0..3); AllGather per (head, token-half) fires after
    qb1 and qb3.
  - out-projection pass(h) runs two head-slots after its AGs fire (AG
    service time is 9-23us with high variance; lag-1 scheduling stalled
    the PE); the 51us of pass(1..3) PE work after attn(3) covers the CC
    stream draining the last three AGs.

Softmax uses exp without max subtraction (logits are O(8) here); denominators
are accumulated on DVE over k-tile pairs then reduced across partitions with
a single ones-matmul, inverted with reciprocal_approx_fast.

Compute is bf16 with fp32 PSUM accumulation; measured l2 rel err vs the fp32
reference is ~5.9e-3.
"""

import numpy as np
import ml_dtypes

B, S, D = 2, 2048, 2048
H, HD = 16, 128
HLOC = 4           # heads per core
CW = HLOC * HD     # 512: per-core v width and out-column width
QB = 4             # q blocks of 512
DT = 16            # d tiles of 128
TB = 4             # token blocks of 512
SCALE = 1.0 / float(np.sqrt(HD))
GROUPS = [[0, 1, 2, 3], [4, 5, 6, 7]]

_cache = {}


def _build():
    import concourse.tile as tile
    import concourse.mybir as mybir
    from concourse import bacc

    BF = mybir.dt.bfloat16
    F32 = mybir.dt.float32

    nc = bacc.Bacc("TRN2", target_bir_lowering=False, debug=False, num_devices=8)

    # Inputs, host-packed partition-major (see _prep_inputs)
    xT = nc.dram_tensor("xT", [128, DT, S], BF, kind="ExternalInput")
    wqk = nc.dram_tensor("wqk", [HLOC, 128, 2 * DT * 128], BF,
                         kind="ExternalInput")
    wv = nc.dram_tensor("wv", [128, DT * CW], BF, kind="ExternalInput")
    bqk = nc.dram_tensor("bqk", [128, 2 * HLOC], F32, kind="ExternalInput")
    bv = nc.dram_tensor("bv", [1, CW], F32, kind="ExternalInput")
    wout = nc.dram_tensor("wout", [128, HLOC * 4 * CW], BF,
                          kind="ExternalInput")
    bout = nc.dram_tensor("bout", [1, CW], F32, kind="ExternalInput")
    out = nc.dram_tensor("out", [S, CW], BF, kind="ExternalOutput")

    # AG buffers: per (head, token-half)
    ag_in = {(h, hf): nc.dram_tensor(f"ag_in{h}_{hf}", [128, 1024], BF,
                                     kind="Internal")
             for h in range(HLOC) for hf in range(2)}
    ag_out = {(h, hf): nc.dram_tensor(f"ag_out{h}_{hf}", [512, 1024], BF,
                                      kind="Internal")
              for h in range(HLOC) for hf in range(2)}

    with tile.TileContext(nc) as tc:
        with (
            tc.tile_pool(name="const", bufs=1) as constp,
            tc.tile_pool(name="pers", bufs=1) as pers,
            tc.tile_pool(name="work", bufs=2) as work,
            tc.tile_pool(name="psum", bufs=2, space="PSUM") as psum,
        ):
            # ---- loads first: the first v-proj chain needs wv chunk 0 and
            # xT(tb0) chunk 0; issue those immediately, interleaved in
            # consumption order (each DMA_DIRECT2D takes ~0.6us to issue) ----
            wv_all = pers.tile([128, DT * CW], BF, name="wv_all")
            xt_all = [work.tile([128, DT * 512], BF, name=f"xt{tb}", tag="xT",
                                bufs=4) for tb in range(TB)]
            bqk_all = constp.tile([128, 2 * HLOC], F32, name="bqk_all")
            bout_sb = constp.tile([1, CW], F32, name="bout_sb")
            bv_sb = constp.tile([1, CW], F32, name="bv_sb")
            for k in range(4):
                cs = slice(k * 4 * CW, (k + 1) * 4 * CW)
                nc.sync.dma_start(wv_all[:, cs], wv[:, cs])
                nc.sync.dma_start(xt_all[0][:, k * 2048:(k + 1) * 2048],
                                  xT[:, 4 * k:4 * (k + 1), 0:512])
                if k == 0:
                    # tiny bias loads on fresh queues before the 2MB loads
                    # monopolize them (a queue's next DMA enqueues only after
                    # its previous one completes)
                    nc.sync.dma_start(bv_sb[:], bv[:])
                    nc.sync.dma_start(bqk_all[:], bqk[:])
                    nc.sync.dma_start(bout_sb[:], bout[:])
            for tb in range(1, TB):
                nc.sync.dma_start(
                    xt_all[tb][:],
                    xT[:, :, tb * 512:(tb + 1) * 512])

            # ---- constants (issued after the hot loads) ----
            ones = constp.tile([128, 1], BF, name="ones")
            nc.gpsimd.memset(ones[:], 1.0)

            # Pair masks for the 4 diagonal k-subtiles, packed two subtiles
            # wide: pairmask[m][:, 512*sub + qq] keeps where
            # qq >= kk + 128*(2m+sub).
            pairmasks = []
            for pm in range(2):
                m = constp.tile([128, 1024], BF, name=f"pmask{pm}",
                                tag=f"pmask{pm}")
                nc.gpsimd.memset(m[:], 1.0)
                for sub in range(2):
                    nc.gpsimd.affine_select(
                        out=m[:, sub * 512:(sub + 1) * 512],
                        in_=m[:, sub * 512:(sub + 1) * 512],
                        compare_op=mybir.AluOpType.is_ge, fill=0.0,
                        base=-128 * (2 * pm + sub), channel_multiplier=-1,
                        pattern=[[1, 512]],
                    )
                pairmasks.append(m)

            bias_bc = constp.tile([128, CW], F32, name="bias_bc")
            nc.gpsimd.partition_broadcast(bias_bc[:], bout_sb[:], channels=128)

            vbias_bc = constp.tile([128, CW], F32, name="vbias_bc")
            nc.gpsimd.partition_broadcast(vbias_bc[:], bv_sb[:], channels=128)

            # ---- persistent v tiles ----
            vt = [pers.tile([128, CW], BF, name=f"v{t}", tag=f"v{t}")
                  for t in range(16)]

            # ---- v projection: v[t] = x @ wv  ([tok, vcol], xT stationary) ----
            for t in range(16):
                tb, j = t // 4, t % 4
                acc = psum.tile([128, CW], F32, name="acc_v", tag="acc", bufs=2)
                for dt in range(DT):
                    c0 = dt * 512
                    nc.tensor.matmul(
                        acc[:],
                        xt_all[tb][:, c0 + j * 128:c0 + (j + 1) * 128],
                        wv_all[:, dt * CW:(dt + 1) * CW],
                        start=(dt == 0), stop=(dt == DT - 1),
                    )
                nc.vector.tensor_tensor(vt[t][:], acc[:], vbias_bc[:],
                                        mybir.AluOpType.add)

            # ---- per-head q/k projection ([col, tok] transposed) ----
            def qk_proj(h):
                wt = work.tile([128, 2 * DT * 128], BF, name=f"wqk{h}",
                               tag="wqk", bufs=2)
                nc.sync.dma_start(wt[:], wqk[h])
                dests = {}
                for qk in range(2):
                    dest = work.tile([128, S], BF, name=f"qkT_{h}_{qk}",
                                     tag="qkT", bufs=4)
                    for tb in range(TB):
                        acc = psum.tile([128, 512], F32, name="acc_qk",
                                        tag="acc", bufs=2)
                        for dt in range(DT):
                            w0 = qk * 2048 + dt * 128
                            nc.tensor.matmul(
                                acc[:], wt[:, w0:w0 + 128],
                                xt_all[tb][:, dt * 512:(dt + 1) * 512],
                                start=(dt == 0), stop=(dt == DT - 1),
                            )
                        bi = 2 * h + qk
                        nc.scalar.activation(
                            dest[:, tb * 512:(tb + 1) * 512], acc[:],
                            mybir.ActivationFunctionType.Identity,
                            bias=bqk_all[:, bi:bi + 1], scale=1.0,
                        )
                    dests[qk] = dest
                return dests[0], dests[1]

            # ---- attention for one head (q-blocks ascending) + AGs ----
            def attention_head(h, qTh, kTh):
                for qb in range(QB):
                    nk = 4 * qb + 4
                    npair = nk // 2
                    y_ps = psum.tile([128, 512], F32, name="y_ps", tag="y")
                    esum = work.tile([128, 1024], BF, name="esum", tag="esum",
                                     bufs=2)
                    prev = None
                    nflush = [0]

                    def flush(prev_pair):
                        e, pr = prev_pair
                        first = nflush[0] == 0
                        last = nflush[0] == npair - 1
                        for s_ in range(2):
                            kt = 2 * pr + s_
                            nc.tensor.matmul(
                                y_ps[:],
                                vt[kt][:, h * 128:(h + 1) * 128],
                                e[:, s_ * 512:(s_ + 1) * 512],
                                start=(first and s_ == 0),
                                stop=(last and s_ == 1),
                            )
                        if first:
                            nc.vector.tensor_copy(esum[:], e[:])
                        else:
                            nc.vector.tensor_tensor(esum[:], esum[:], e[:],
                                                    mybir.AluOpType.add)
                        nflush[0] += 1

                    for pr in range(npair):
                        sc = psum.tile([128, 1024], F32, name="sc", tag="s",
                                       bufs=2)
                        for s_ in range(2):
                            kt = 2 * pr + s_
                            nc.tensor.matmul(
                                sc[:, s_ * 512:(s_ + 1) * 512],
                                kTh[:, kt * 128:(kt + 1) * 128],
                                qTh[:, qb * 512:(qb + 1) * 512],
                                start=True, stop=True,
                            )
                        e = work.tile([128, 1024], BF, name="expT", tag="expT",
                                      bufs=4)
                        nc.scalar.activation(
                            e[:], sc[:], mybir.ActivationFunctionType.Exp,
                            scale=SCALE,
                        )
                        pm = pr - (npair - 2)
                        if pm >= 0:
                            nc.vector.tensor_tensor(e[:], e[:],
                                                    pairmasks[pm][:],
                                                    mybir.AluOpType.mult)
                        if prev is not None:
                            flush(prev)
                        prev = (e, pr)
                    flush(prev)

                    esum_f = work.tile([128, 512], BF, name="esum_f",
                                       tag="esum_f", bufs=2)
                    nc.vector.tensor_tensor(esum_f[:], esum[:, 0:512],
                                            esum[:, 512:1024],
                                            mybir.AluOpType.add)
                    sum_ps = psum.tile([1, 512], F32, name="sum_ps", tag="y")
                    nc.tensor.matmul(sum_ps[:], ones[:], esum_f[:],
                                     start=True, stop=True)
                    recip = work.tile([1, 512], F32, name="recip", tag="recip",
                                      bufs=2)
                    nc.vector.reciprocal_approx_fast(recip[:], sum_ps[:])
                    rbc = work.tile([128, 512], F32, name="rbc", tag="rbc",
                                    bufs=2)
                    nc.gpsimd.partition_broadcast(rbc[:], recip[:], channels=128)
                    ynorm = work.tile([128, 512], BF, name="ynorm", tag="ynorm",
                                      bufs=3)
                    nc.vector.tensor_tensor(ynorm[:], y_ps[:], rbc[:],
                                            mybir.AluOpType.mult)
                    hf, co = qb // 2, (qb % 2) * 512
                    nc.sync.dma_start(
                        ag_in[(h, hf)][:, co:co + 512], ynorm[:])
                    if qb in (1, 3):
                        nc.gpsimd.collective_compute(
                            "AllGather", mybir.AluOpType.bypass,
                            replica_groups=GROUPS,
                            ins=[ag_in[(h, hf)].ap()],
                            outs=[ag_out[(h, hf)].ap()],
                        )

            # ---- out-projection partial pass for head-chunk h ----
            wout_all = pers.tile([128, HLOC * 4 * CW], BF, name="wout_all")

            def load_wout():
                nc.sync.dma_start(wout_all[:], wout[:])

            part = {}

            def outproj_pass(h):
                for tc_ in range(4):
                    hf, co = tc_ // 2, (tc_ % 2) * 512
                    ygt = []
                    for i in range(4):
                        t = work.tile([128, 512], BF, name=f"yg_{h}_{tc_}_{i}",
                                      tag="ygt", bufs=6)
                        nc.sync.dma_start(
                            t[:], ag_out[(h, hf)][i * 128:(i + 1) * 128,
                                                  co:co + 512])
                        ygt.append(t)
                    for j in range(4):
                        t = tc_ * 4 + j
                        acc = psum.tile([128, CW], F32, name="acc_o",
                                        tag="acc", bufs=2)
                        for i in range(4):
                            w0 = (h * 4 + i) * CW
                            nc.tensor.matmul(
                                acc[:],
                                ygt[i][:, j * 128:(j + 1) * 128],
                                wout_all[:, w0:w0 + CW],
                                start=(i == 0), stop=(i == 3),
                            )
                        if h == 0:
                            p = work.tile([128, CW], BF, name=f"part{t}",
                                          tag=f"part{t}", bufs=1)
                            part[t] = p
                            nc.vector.tensor_tensor(p[:], acc[:], bias_bc[:],
                                                    mybir.AluOpType.add)
                        elif h < HLOC - 1:
                            nc.vector.tensor_tensor(part[t][:], part[t][:],
                                                    acc[:],
                                                    mybir.AluOpType.add)
                        else:
                            osb = work.tile([128, CW], BF, name="osb",
                                            tag="osb", bufs=2)
                            nc.vector.tensor_tensor(osb[:], part[t][:], acc[:],
                                                    mybir.AluOpType.add)
                            nc.sync.dma_start(
                                out[t * 128:(t + 1) * 128, :], osb[:])

            # ---- head pipeline: pass(h) two slots behind its AGs ----
            qk_tiles = qk_proj(0)
            load_wout()
            for h in range(HLOC):
                attention_head(h, *qk_tiles)
                qk_tiles = qk_proj(h + 1) if h + 1 < HLOC else None
                if h > 1:
                    outproj_pass(h - 2)
            outproj_pass(HLOC - 2)
            outproj_pass(HLOC - 1)

    nc.compile()
    return nc


def _prep_inputs(x, w_qkv, b_qkv, w_out, b_out):
    """Host-side sharding + partition-major packing. in_maps for 8 cores."""
    bf16 = ml_dtypes.bfloat16
    x = np.asarray(x, dtype=np.float32)
    w_qkv = np.asarray(w_qkv, dtype=np.float32)
    b_qkv = np.asarray(b_qkv, dtype=np.float32)
    w_out = np.asarray(w_out, dtype=np.float32)
    b_out = np.asarray(b_out, dtype=np.float32)

    # xT packed [p, dt, tok]: xTp[p, dt, c] = x[b].T[dt*128+p, c]
    xT_b = [np.ascontiguousarray(
                x[b].T.reshape(DT, 128, S).transpose(1, 0, 2)).astype(bf16)
            for b in range(B)]

    in_maps = []
    for c in range(8):
        b, g = c // 4, c % 4
        cols = slice(CW * g, CW * (g + 1))

        # wqk packed [p, qk, dt, col] per head -> [HLOC, 128, 2*DT*128]
        wqk = np.empty((HLOC, 2, DT, 128, 128), np.float32)
        bqk = np.empty((HLOC, 2, 128), np.float32)
        for h in range(HLOC):
            gh = 4 * g + h
            for qk in range(2):
                wcol = w_qkv[:, qk * D + 128 * gh: qk * D + 128 * (gh + 1)]
                wqk[h, qk] = wcol.reshape(DT, 128, 128)
                bqk[h, qk] = b_qkv[qk * D + 128 * gh: qk * D + 128 * (gh + 1)]
        wqk_p = wqk.transpose(0, 3, 1, 2, 4).reshape(HLOC, 128, 2 * DT * 128)
        # bqk packed [p, 2h+qk]
        bqk_p = bqk.transpose(2, 0, 1).reshape(128, 2 * HLOC)

        # wv packed [p, dt, vcol]
        wv_ = w_qkv[:, 2 * D:3 * D][:, cols]
        wv_p = wv_.reshape(DT, 128, CW).transpose(1, 0, 2).reshape(128, DT * CW)
        bv_ = b_qkv[2 * D:3 * D][cols]

        # w_out rows permuted to AG rank-major order, packed [p, h, i, col]
        wout_loc = w_out[:, cols]
        wout_t = np.empty((HLOC, 4, 128, CW), np.float32)
        for h in range(HLOC):
            for i in range(4):
                wout_t[h, i] = wout_loc[512 * i + 128 * h: 512 * i + 128 * (h + 1), :]
        wout_p = wout_t.transpose(2, 0, 1, 3).reshape(128, HLOC * 4 * CW)

        in_maps.append({
            "xT": xT_b[b],
            "wqk": np.ascontiguousarray(wqk_p).astype(bf16),
            "wv": np.ascontiguousarray(wv_p).astype(bf16),
            "bqk": np.ascontiguousarray(bqk_p),
            "bv": np.ascontiguousarray(bv_.reshape(1, CW)),
            "wout": np.ascontiguousarray(wout_p).astype(bf16),
            "bout": np.ascontiguousarray(b_out[cols].reshape(1, CW)),
        })
    return in_maps


def kernel(x, w_qkv, b_qkv, w_out, b_out, _trace=False, _trace_kwargs=None):
    from concourse.bass_utils import run_bass_kernel_spmd

    if "nc" not in _cache:
        _cache["nc"] = _build()
    nc = _cache["nc"]

    in_maps = _prep_inputs(x, w_qkv, b_qkv, w_out, b_out)
    res = run_bass_kernel_spmd(
        nc, in_maps, core_ids=list(range(8)),
        trace=_trace, **(_trace_kwargs or {}),
    )

    out = np.empty((B, S, D), dtype=np.float32)
    for c in range(8):
        b, g = c // 4, c % 4
        out[b][:, CW * g:CW * (g + 1)] = np.asarray(
            res.results[c]["out"], dtype=np.float32)
    kernel.last_result = res
    return out


# revision 19
# speedup vs baseline: 1.0348x; 1.0114x over previous
"""Causal self-attention (B=2, S=2048, D=2048, H=16) on 8 TRN2 NeuronCores.

Sharding: 2 batches x 4 head-groups.  Core c handles batch c//4 and heads
[4*(c%4) .. 4*(c%4)+3]; each core produces output columns [512*(c%4) ...].

Schedule (per core), tuned from the 447us-baseline trace:
  - DMA issue order interleaves (wv[dt], xT[dt, tb0]) so the first v-proj
    accumulation starts ~2us after DMA-subsystem spin-up (a coarse wait
    cost the old baseline ~28us of prologue).
  - q-blocks ascend (0..3); AllGather per (head, token-half) fires after
    qb1 and qb3.
  - out-projection pass(h) runs two head-slots after its AGs fire (AG
    service time is 9-24us with high variance; lag-1 scheduling stalled
    the PE); the 51us of pass(1..3) PE work after attn(3) covers the CC
    stream draining the last AGs.
  - output rows are written bf16 (error budget is ample) to halve the
    final DMA drain.

Softmax uses exp without max subtraction (logits are O(8) here); denominators
are accumulated on DVE over k-tile pairs then reduced across partitions with
a single ones-matmul, inverted with reciprocal_approx_fast.

Compute is bf16 with fp32 PSUM accumulation; measured l2 rel err vs the fp32
reference is ~6.1e-3.
"""

import numpy as np
import ml_dtypes

B, S, D = 2, 2048, 2048
H, HD = 16, 128
HLOC = 4           # heads per core
CW = HLOC * HD     # 512: per-core v width and out-column width
QB = 4             # q blocks of 512
DT = 16            # d tiles of 128
TB = 4             # token blocks of 512
SCALE = 1.0 / float(np.sqrt(HD))
GROUPS = [[0, 1, 2, 3], [4, 5, 6, 7]]

_cache = {}


def _build():
    import concourse.tile as tile
    import concourse.mybir as mybir
    from concourse import bacc

    BF = mybir.dt.bfloat16
    F32 = mybir.dt.float32

    nc = bacc.Bacc("TRN2", target_bir_lowering=False, debug=False, num_devices=8)

    # Inputs (per-core shards, host-prepared)
    xT = nc.dram_tensor("xT", [D, S], BF, kind="ExternalInput")          # x[batch].T
    wqk = nc.dram_tensor("wqk", [HLOC, 2, DT, 128, 128], BF, kind="ExternalInput")
    wv = nc.dram_tensor("wv", [DT, 128, CW], BF, kind="ExternalInput")
    bqk = nc.dram_tensor("bqk", [HLOC, 2, 128, 1], F32, kind="ExternalInput")
    bv = nc.dram_tensor("bv", [1, CW], F32, kind="ExternalInput")
    # w_out rows permuted: wout[h][i] = w_out[512*i + 128*h : +128, cols]
    wout = nc.dram_tensor("wout", [HLOC, 4, 128, CW], BF, kind="ExternalInput")
    bout = nc.dram_tensor("bout", [1, CW], F32, kind="ExternalInput")
    out = nc.dram_tensor("out", [S, CW], BF, kind="ExternalOutput")

    # per (head, token-half) AG buffers; half hf covers tokens
    # [1024*hf, 1024*(hf+1)) = q-blocks {2hf, 2hf+1}
    ag_in = {(h, hf): nc.dram_tensor(f"ag_in{h}_{hf}", [128, 1024], BF,
                                     kind="Internal")
             for h in range(HLOC) for hf in range(2)}
    ag_out = {(h, hf): nc.dram_tensor(f"ag_out{h}_{hf}", [512, 1024], BF,
                                      kind="Internal")
              for h in range(HLOC) for hf in range(2)}

    with tile.TileContext(nc) as tc:
        with (
            tc.tile_pool(name="const", bufs=1) as constp,
            tc.tile_pool(name="pers", bufs=1) as pers,
            tc.tile_pool(name="work", bufs=2) as work,
            tc.tile_pool(name="psum", bufs=2, space="PSUM") as psum,
        ):
            # ---- constants ----
            ones = constp.tile([128, 1], BF, name="ones")
            nc.gpsimd.memset(ones[:], 1.0)

            # Pair masks for the 4 diagonal k-subtiles, packed two subtiles
            # wide: pairmask[m][:, 512*sub + qq] keeps where
            # qq >= kk + 128*(2m+sub).
            pairmasks = []
            for pm in range(2):
                m = constp.tile([128, 1024], BF, name=f"pmask{pm}",
                                tag=f"pmask{pm}")
                nc.gpsimd.memset(m[:], 1.0)
                for sub in range(2):
                    nc.gpsimd.affine_select(
                        out=m[:, sub * 512:(sub + 1) * 512],
                        in_=m[:, sub * 512:(sub + 1) * 512],
                        compare_op=mybir.AluOpType.is_ge, fill=0.0,
                        base=-128 * (2 * pm + sub), channel_multiplier=-1,
                        pattern=[[1, 512]],
                    )
                pairmasks.append(m)

            bout_sb = constp.tile([1, CW], F32, name="bout_sb")
            nc.sync.dma_start(bout_sb[:], bout[:])
            bias_bc = constp.tile([128, CW], F32, name="bias_bc")
            nc.gpsimd.partition_broadcast(bias_bc[:], bout_sb[:], channels=128)

            bv_sb = constp.tile([1, CW], F32, name="bv_sb")
            nc.sync.dma_start(bv_sb[:], bv[:])
            vbias_bc = constp.tile([128, CW], F32, name="vbias_bc")
            nc.gpsimd.partition_broadcast(vbias_bc[:], bv_sb[:], channels=128)

            bqk_sb = {}
            for h in range(HLOC):
                for qk in range(2):
                    t = constp.tile([128, 1], F32, name=f"bqk{h}{qk}",
                                    tag=f"bqk{h}{qk}")
                    nc.sync.dma_start(t[:], bqk[h, qk])
                    bqk_sb[(h, qk)] = t

            # ---- persistent v tiles ----
            vt = [pers.tile([128, CW], BF, name=f"v{t}", tag=f"v{t}")
                  for t in range(16)]

            # ---- loads: (wv[dt], xT[dt,tb0]) interleaved so the first
            # v-proj accumulation chain is gated only on its own tiles,
            # then the remaining token blocks ----
            wv_sb = []
            xt_tiles = {}

            def load_xt(dt, tb):
                t = work.tile([128, 512], BF, name=f"xt_{dt}_{tb}",
                              tag="xT", bufs=68)
                nc.sync.dma_start(
                    t[:], xT[dt * 128:(dt + 1) * 128, tb * 512:(tb + 1) * 512])
                xt_tiles[(dt, tb)] = t

            for dt in range(DT):
                wvp = work.tile([128, CW], BF, name=f"wvp{dt}", tag="p512",
                                bufs=17)
                nc.sync.dma_start(wvp[:], wv[dt])
                wv_sb.append(wvp)
                load_xt(dt, 0)
            for tb in range(1, TB):
                for dt in range(DT):
                    load_xt(dt, tb)

            # ---- v projection: v[t] = x @ wv  ([tok, vcol], xT stationary) ----
            for t in range(16):
                tb, j = t // 4, t % 4
                acc = psum.tile([128, CW], F32, name="acc_v", tag="acc", bufs=2)
                for dt in range(DT):
                    nc.tensor.matmul(
                        acc[:],
                        xt_tiles[(dt, tb)][:, j * 128:(j + 1) * 128],
                        wv_sb[dt][:],
                        start=(dt == 0), stop=(dt == DT - 1),
                    )
                nc.vector.tensor_tensor(vt[t][:], acc[:], vbias_bc[:],
                                        mybir.AluOpType.add)

            # ---- per-head q/k projection ([col, tok] transposed) ----
            def qk_proj(h):
                dests = {}
                for qk in range(2):
                    wts = []
                    for dt in range(DT):
                        wt = work.tile([128, 128], BF, name=f"w_{h}_{qk}_{dt}",
                                       tag="w", bufs=24)
                        nc.sync.dma_start(wt[:], wqk[h, qk, dt])
                        wts.append(wt)
                    dest = work.tile([128, S], BF, name=f"qkT_{h}_{qk}",
                                     tag="qkT", bufs=4)
                    for tb in range(TB):
                        acc = psum.tile([128, 512], F32, name="acc_qk",
                                        tag="acc", bufs=2)
                        for dt in range(DT):
                            nc.tensor.matmul(
                                acc[:], wts[dt][:], xt_tiles[(dt, tb)][:],
                                start=(dt == 0), stop=(dt == DT - 1),
                            )
                        nc.scalar.activation(
                            dest[:, tb * 512:(tb + 1) * 512], acc[:],
                            mybir.ActivationFunctionType.Identity,
                            bias=bqk_sb[(h, qk)][:], scale=1.0,
                        )
                    dests[qk] = dest
                return dests[0], dests[1]

            # ---- attention for one head (q-blocks ascending) + per-half AG ----
            def attention_head(h, qTh, kTh):
                for qb in range(QB):
                    nk = 4 * qb + 4
                    npair = nk // 2
                    y_ps = psum.tile([128, 512], F32, name="y_ps", tag="y")
                    esum = work.tile([128, 1024], BF, name="esum", tag="esum",
                                     bufs=2)
                    prev = None
                    nflush = [0]

                    def flush(prev_pair):
                        e, pr = prev_pair
                        first = nflush[0] == 0
                        last = nflush[0] == npair - 1
                        for s_ in range(2):
                            kt = 2 * pr + s_
                            nc.tensor.matmul(
                                y_ps[:],
                                vt[kt][:, h * 128:(h + 1) * 128],
                                e[:, s_ * 512:(s_ + 1) * 512],
                                start=(first and s_ == 0),
                                stop=(last and s_ == 1),
                            )
                        if first:
                            nc.vector.tensor_copy(esum[:], e[:])
                        else:
                            nc.vector.tensor_tensor(esum[:], esum[:], e[:],
                                                    mybir.AluOpType.add)
                        nflush[0] += 1

                    for pr in range(npair):
                        sc = psum.tile([128, 1024], F32, name="sc", tag="s",
                                       bufs=2)
                        for s_ in range(2):
                            kt = 2 * pr + s_
                            nc.tensor.matmul(
                                sc[:, s_ * 512:(s_ + 1) * 512],
                                kTh[:, kt * 128:(kt + 1) * 128],
                                qTh[:, qb * 512:(qb + 1) * 512],
                                start=True, stop=True,
                            )
                        e = work.tile([128, 1024], BF, name="expT", tag="expT",
                                      bufs=4)
                        nc.scalar.activation(
                            e[:], sc[:], mybir.ActivationFunctionType.Exp,
                            scale=SCALE,
                        )
                        pm = pr - (npair - 2)
                        if pm >= 0:
                            nc.vector.tensor_tensor(e[:], e[:],
                                                    pairmasks[pm][:],
                                                    mybir.AluOpType.mult)
                        if prev is not None:
                            flush(prev)
                        prev = (e, pr)
                    flush(prev)

                    esum_f = work.tile([128, 512], BF, name="esum_f",
                                       tag="esum_f", bufs=2)
                    nc.vector.tensor_tensor(esum_f[:], esum[:, 0:512],
                                            esum[:, 512:1024],
                                            mybir.AluOpType.add)
                    sum_ps = psum.tile([1, 512], F32, name="sum_ps", tag="y")
                    nc.tensor.matmul(sum_ps[:], ones[:], esum_f[:],
                                     start=True, stop=True)
                    recip = work.tile([1, 512], F32, name="recip", tag="recip",
                                      bufs=2)
                    nc.vector.reciprocal_approx_fast(recip[:], sum_ps[:])
                    rbc = work.tile([128, 512], F32, name="rbc", tag="rbc",
                                    bufs=2)
                    nc.gpsimd.partition_broadcast(rbc[:], recip[:], channels=128)
                    ynorm = work.tile([128, 512], BF, name="ynorm", tag="ynorm",
                                      bufs=3)
                    nc.vector.tensor_tensor(ynorm[:], y_ps[:], rbc[:],
                                            mybir.AluOpType.mult)
                    hf, co = qb // 2, (qb % 2) * 512
                    nc.sync.dma_start(
                        ag_in[(h, hf)][:, co:co + 512], ynorm[:])
                    if qb in (1, 3):
                        nc.gpsimd.collective_compute(
                            "AllGather", mybir.AluOpType.bypass,
                            replica_groups=GROUPS,
                            ins=[ag_in[(h, hf)].ap()],
                            outs=[ag_out[(h, hf)].ap()],
                        )

            # ---- out-projection partial pass for head-chunk h ----
            wout_sb = {}

            def load_wout():
                for h in range(HLOC):
                    for i in range(4):
                        t = work.tile([128, CW], BF, name=f"wout{h}{i}",
                                      tag="p512", bufs=17)
                        nc.sync.dma_start(t[:], wout[h, i])
                        wout_sb[(h, i)] = t

            part = {}

            def outproj_pass(h):
                for tc_ in range(4):
                    hf, co = tc_ // 2, (tc_ % 2) * 512
                    ygt = []
                    for i in range(4):
                        t = work.tile([128, 512], BF, name=f"yg_{h}_{tc_}_{i}",
                                      tag="ygt", bufs=10)
                        nc.sync.dma_start(
                            t[:], ag_out[(h, hf)][i * 128:(i + 1) * 128,
                                                  co:co + 512])
                        ygt.append(t)
                    for j in range(4):
                        t = tc_ * 4 + j
                        acc = psum.tile([128, CW], F32, name="acc_o",
                                        tag="acc", bufs=2)
                        for i in range(4):
                            nc.tensor.matmul(
                                acc[:],
                                ygt[i][:, j * 128:(j + 1) * 128],
                                wout_sb[(h, i)][:],
                                start=(i == 0), stop=(i == 3),
                            )
                        if h == 0:
                            p = work.tile([128, CW], BF, name=f"part{t}",
                                          tag=f"part{t}", bufs=1)
                            part[t] = p
                            nc.vector.tensor_tensor(p[:], acc[:], bias_bc[:],
                                                    mybir.AluOpType.add)
                        elif h < HLOC - 1:
                            nc.vector.tensor_tensor(part[t][:], part[t][:],
                                                    acc[:],
                                                    mybir.AluOpType.add)
                        else:
                            osb = work.tile([128, CW], BF, name="osb",
                                            tag="osb", bufs=3)
                            nc.vector.tensor_tensor(osb[:], part[t][:], acc[:],
                                                    mybir.AluOpType.add)
                            nc.sync.dma_start(
                                out[t * 128:(t + 1) * 128, :], osb[:])

            # ---- head pipeline: pass(h) two slots behind its AGs ----
            qk_tiles = qk_proj(0)
            load_wout()
            for h in range(HLOC):
                attention_head(h, *qk_tiles)
                qk_tiles = qk_proj(h + 1) if h + 1 < HLOC else None
                if h > 1:
                    outproj_pass(h - 2)
            outproj_pass(HLOC - 2)
            outproj_pass(HLOC - 1)

    nc.compile()
    return nc


def _prep_inputs(x, w_qkv, b_qkv, w_out, b_out):
    """Host-side sharding/layout. Returns in_maps for the 8 cores."""
    bf16 = ml_dtypes.bfloat16
    x = np.asarray(x, dtype=np.float32)
    w_qkv = np.asarray(w_qkv, dtype=np.float32)
    b_qkv = np.asarray(b_qkv, dtype=np.float32)
    w_out = np.asarray(w_out, dtype=np.float32)
    b_out = np.asarray(b_out, dtype=np.float32)

    xT_b = [np.ascontiguousarray(x[b].T).astype(bf16) for b in range(B)]

    in_maps = []
    for c in range(8):
        b, g = c // 4, c % 4
        cols = slice(CW * g, CW * (g + 1))

        # wqk[h][0]=q, [1]=k tiles for global head 4g+h, [dt, 128, 128]
        wqk = np.empty((HLOC, 2, DT, 128, 128), np.float32)
        bqk = np.empty((HLOC, 2, 128, 1), np.float32)
        for h in range(HLOC):
            gh = 4 * g + h
            for qk in range(2):
                wcol = w_qkv[:, qk * D + 128 * gh: qk * D + 128 * (gh + 1)]
                wqk[h, qk] = wcol.reshape(DT, 128, 128)
                bqk[h, qk, :, 0] = b_qkv[qk * D + 128 * gh: qk * D + 128 * (gh + 1)]

        wv_ = w_qkv[:, 2 * D:3 * D][:, cols]
        bv_ = b_qkv[2 * D:3 * D][cols]

        # w_out rows permuted to the AG's rank-major order per head chunk
        wout_loc = w_out[:, cols]
        wout_t = np.empty((HLOC, 4, 128, CW), np.float32)
        for h in range(HLOC):
            for i in range(4):
                wout_t[h, i] = wout_loc[512 * i + 128 * h: 512 * i + 128 * (h + 1), :]

        in_maps.append({
            "xT": xT_b[b],
            "wqk": np.ascontiguousarray(wqk).astype(bf16),
            "wv": np.ascontiguousarray(wv_.reshape(DT, 128, CW)).astype(bf16),
            "bqk": np.ascontiguousarray(bqk),
            "bv": np.ascontiguousarray(bv_.reshape(1, CW)),
            "wout": np.ascontiguousarray(wout_t).astype(bf16),
            "bout": np.ascontiguousarray(b_out[cols].reshape(1, CW)),
        })
    return in_maps


def kernel(x, w_qkv, b_qkv, w_out, b_out, _trace=False, _trace_kwargs=None):
    from concourse.bass_utils import run_bass_kernel_spmd

    if "nc" not in _cache:
        _cache["nc"] = _build()
    nc = _cache["nc"]

    in_maps = _prep_inputs(x, w_qkv, b_qkv, w_out, b_out)
    res = run_bass_kernel_spmd(
        nc, in_maps, core_ids=list(range(8)),
        trace=_trace, **(_trace_kwargs or {}),
    )

    out = np.empty((B, S, D), dtype=np.float32)
    for c in range(8):
        b, g = c // 4, c % 4
        out[b][:, CW * g:CW * (g + 1)] = np.asarray(
            res.results[c]["out"], dtype=np.float32)
    kernel.last_result = res
    return out
